# revision 1
# baseline (speedup 1.0000x reference)
"""Trainium2 Bass kernel for causal multi-head attention.

Problem: X[2, 2048, 1024] fp32, 16 heads x head_dim 64, causal softmax,
out = softmax(QK^T/sqrt(64)) V @ Wo + bo.

Sharding: tensor-parallel over heads. Each of the 8 cores gets 2 heads:
Wq/Wk/Wv column slices [1024, 128], Wo row slice [128, 1024]. X replicated,
pre-transposed and cast to bf16 on the host ([B, D, N]). Each core returns
its partial output [2, 2048, 1024]; host sums partials + bias.

Per-core dataflow (per batch b):
  X^T[128, 8, 2048] bf16    <- direct DMA (host pre-transposed)
  Q^T,K^T[128, 2048] f32r   <- Wq/Wk chunk.T @ X^T  (bf16 MMs, fp32 PSUM)
  V^T[128, 2048] bf16       <- Wv chunk.T @ X^T, then PE-transposed into
  V[k, 130] bf16               natural layout (+ ones cols for softmax sums)
  S^T[k, q] (f32r MMs)      <- K^T chunk (lhsT) x Q^T (rhs), 2 heads
                               row-packed, causal block staircase only
  expS^T bf16               <- ACT exp(scale*S), diag blocks masked
  ctx^T+sums[65, q]         <- Vaug[k,65] (lhsT) x expS^T; row 64 = sums
  normalize                 <- gpsimd partition_broadcast(sums) + fast
                               reciprocal + fused multiply into ctxT (f32r)
  O[q, 1024] fp32           <- ctx^T (lhsT, f32r) x Wo (f32r), DMA out

Note: no on-device DMA-XBAR transposes anywhere — f32r matmul weight loads
use the XBAR and corrupt concurrent DMA transposes (hardware conflict,
verified empirically).
"""

from contextlib import ExitStack

import numpy as np
import ml_dtypes

import concourse.bass as bass
import concourse.mybir as mybir
import concourse.tile as tile
from concourse import bacc
from concourse.bass_utils import run_bass_kernel_spmd
from concourse.masks import make_upper_triangular, make_identity

B, N, D = 2, 2048, 1024
H, DH = 16, 64
NCORES = 8
HPC = H // NCORES            # heads per core = 2
DPC = HPC * DH               # projection cols per core = 128
P = 128
QG = 512                     # q-column group width
NQG = N // QG                # 4
NKT = N // P                 # 16
NQT = N // P                 # 16
DC = D // P                  # 8 contraction chunks
VW = 2 * DH + 2              # V tile width: h0 | ones | h1 | ones = 130
SCALE = 1.0 / float(np.sqrt(DH))

BF16 = mybir.dt.bfloat16
F32 = mybir.dt.float32
F32R = mybir.dt.float32r


def build_nc(
    xt_split: int = 4,
    ps_big_bufs: int = 2,
    ps_s_bufs: int = 3,
    ps_ctx_bufs: int = 3,
    exps_bufs: int = 10,
    o_bufs: int = 4,
    o_copy_split: bool = False,
    schedule=None,
) -> bass.Bass:
    if schedule is None:
        schedule = [("qkv", 0), ("qkv", 1)] + [
            ("attn", b, qg) for b in range(B) for qg in range(NQG)]
    nc = bacc.Bacc("TRN2", target_bir_lowering=False, debug=False)

    # X^T pre-transposed on host: [B, D, N] bf16
    x = nc.dram_tensor("x", [B, D, N], BF16, kind="ExternalInput")
    wq = nc.dram_tensor("wq", [D, DPC], BF16, kind="ExternalInput")
    wk = nc.dram_tensor("wk", [D, DPC], BF16, kind="ExternalInput")
    wv = nc.dram_tensor("wv", [D, DPC], BF16, kind="ExternalInput")
    wo = nc.dram_tensor("wo", [DPC, D], F32R, kind="ExternalInput")
    out = nc.dram_tensor("out", [B, N, D], F32, kind="ExternalOutput")

    with tile.TileContext(nc) as tc, ExitStack() as ctx:
        consts = ctx.enter_context(tc.tile_pool(name="consts", bufs=1))
        xt_pool = ctx.enter_context(tc.tile_pool(name="xt", bufs=2))
        qk_pool = ctx.enter_context(tc.tile_pool(name="qk", bufs=1))
        vt_pool = ctx.enter_context(tc.tile_pool(name="vt", bufs=2))
        v_pool = ctx.enter_context(tc.tile_pool(name="v", bufs=1))
        ctx_pool = ctx.enter_context(tc.tile_pool(name="ctx", bufs=1))
        exps_pool = ctx.enter_context(tc.tile_pool(name="exps", bufs=exps_bufs))
        sums_pool = ctx.enter_context(tc.tile_pool(name="sums", bufs=3))
        rcp_pool = ctx.enter_context(tc.tile_pool(name="rcp", bufs=3))
        o_pool = ctx.enter_context(tc.tile_pool(name="o", bufs=o_bufs))

        ps_big = ctx.enter_context(
            tc.tile_pool(name="ps_big", bufs=ps_big_bufs, space="PSUM"))
        ps_s = ctx.enter_context(
            tc.tile_pool(name="ps_s", bufs=ps_s_bufs, space="PSUM"))
        ps_ctx = ctx.enter_context(
            tc.tile_pool(name="ps_ctx", bufs=ps_ctx_bufs, space="PSUM"))

        # --- constants ---
        wq_sb = consts.tile([P, DC, DPC], BF16, tag="wq")
        wk_sb = consts.tile([P, DC, DPC], BF16, tag="wk")
        wv_sb = consts.tile([P, DC, DPC], BF16, tag="wv")
        wo_sb = consts.tile([P, D], F32R, tag="wo")
        # weights on the ACT HWDGE queue so they flow in parallel with the
        # X^T loads on the SP queue
        nc.scalar.dma_start(wq_sb[:], wq[:].rearrange("(dc p) m -> p dc m", p=P))
        nc.scalar.dma_start(wk_sb[:], wk[:].rearrange("(dc p) m -> p dc m", p=P))
        nc.scalar.dma_start(wv_sb[:], wv[:].rearrange("(dc p) m -> p dc m", p=P))
        nc.scalar.dma_start(wo_sb[:], wo[:])

        # mask[k, q] = 1.0 if k <= q else 0   (for S^T layout diagonal blocks)
        mask_sb = consts.tile([P, P], BF16, tag="mask")
        make_upper_triangular(nc, mask_sb[:], val=1.0, diag=True)
        ident_sb = consts.tile([P, P], BF16, tag="ident")
        make_identity(nc, ident_sb[:])
        ones_sb = consts.tile([P, DH], F32R, tag="ones")
        nc.gpsimd.memset(ones_sb[:].bitcast(mybir.dt.uint32), 0x3F800000)

        # --- X^T for both batches loaded up front (pre-transposed on host) ---
        xts = []
        for b in range(B):
            xt = xt_pool.tile([P, DC, N], BF16, tag="xt", name=f"xt{b}")
            xts.append(xt)
            xrr = x[b].rearrange("(dc p) n -> p dc n", p=P)
            xw = N // xt_split
            for i in range(xt_split):
                if b == 0 and i == 0:
                    # fine-grained first chunk: the first QKV matmul only
                    # needs one dc block, so let it start after 128KB
                    for dc in range(DC):
                        nc.sync.dma_start(
                            xt[:, dc, 0:xw], xrr[:, dc, 0:xw])
                else:
                    nc.sync.dma_start(
                        xt[:, :, i * xw:(i + 1) * xw], xrr[:, :, i * xw:(i + 1) * xw])

        qTs, kTs, vs, ctxTs = {}, {}, {}, {}

        def emit_qkv(b):
            xt = xts[b]
            # Q^T / K^T (f32r) and V^T (bf16) projections: [128, N]
            qT = qk_pool.tile([P, N], F32R, tag=f"qT{b}", name=f"qT{b}")
            kT = qk_pool.tile([P, N], F32R, tag=f"kT{b}", name=f"kT{b}")
            vT = vt_pool.tile([P, N], BF16, tag="vT", name=f"vT{b}")
            qTs[b], kTs[b] = qT, kT
            for qg in range(NQG):
                sl = slice(qg * QG, (qg + 1) * QG)
                for w_sb, dstT in ((wq_sb, qT), (wk_sb, kT), (wv_sb, vT)):
                    ps = ps_big.tile([P, QG], F32, tag="ps_big", name="ps")
                    for dc in range(DC):
                        nc.tensor.matmul(
                            ps[:], w_sb[:, dc], xt[:, dc, sl],
                            start=(dc == 0), stop=(dc == DC - 1),
                        )
                    nc.vector.tensor_copy(dstT[:, sl], ps[:])

            # V natural layout [k-part, kt, 130] bf16 with ones cols
            v = v_pool.tile([P, NKT, VW], BF16, tag=f"v{b}", name=f"v{b}")
            vs[b] = v
            nc.gpsimd.memset(v[:, :, DH], 1.0)
            nc.gpsimd.memset(v[:, :, 2 * DH + 1], 1.0)
            for kt in range(NKT):
                ps_vt = ps_s.tile([P, P], BF16, tag="ps_s", name="ps_vt")
                nc.tensor.transpose(
                    ps_vt[:], vT[:, kt * P:(kt + 1) * P], ident_sb[:]
                )
                nc.vector.tensor_copy(v[:, kt, 0:DH], ps_vt[:, 0:DH])
                nc.vector.tensor_copy(v[:, kt, DH + 1:2 * DH + 1], ps_vt[:, DH:2 * DH])

        def emit_attn(b, qg):
            qT, kT, v = qTs[b], kTs[b], vs[b]
            if b not in ctxTs:
                ctxTs[b] = ctx_pool.tile([P, N], F32R, tag=f"ctxT{b}", name=f"ctxT{b}")
            ctxT = ctxTs[b]
            nkt = 4 * qg + 4  # k tiles for this q group (causal)
            pc = [
                ps_ctx.tile([DH + 1, QG], F32, tag="ps_ctx", name=f"pc{h}")
                for h in range(HPC)
            ]
            for kt in range(nkt):
                cs = max(0, kt * P - qg * QG)  # valid col start (diag staircase)
                ws = min(cs, QG - 256)  # widen short diag MMs to 256 (f32r)
                es = []
                pss = []
                for h in range(HPC):
                    ps_sc = ps_s.tile([P, QG], F32, tag="ps_s", name="ps_sc")
                    pss.append(ps_sc)
                    nc.tensor.matmul(
                        ps_sc[:, 0:QG - ws],
                        kT[64 * h:64 * h + 64, kt * P:(kt + 1) * P],
                        qT[64 * h:64 * h + 64, qg * QG + ws:(qg + 1) * QG],
                        start=True, stop=True,
                        tile_position=(64 * h, 0),
                    )
                for h in range(HPC):
                    e = exps_pool.tile([P, QG], BF16, tag="exps", name="e")
                    es.append(e)
                    nc.scalar.activation(
                        e[:, cs:QG], pss[h][:, cs - ws:QG - ws],
                        mybir.ActivationFunctionType.Exp, scale=SCALE,
                    )
                    if kt * P >= qg * QG:
                        # diagonal 128-block: keep (q_rel - k) >= 0, on gpsimd
                        nc.gpsimd.affine_select(
                            out=e[:, cs:cs + P], in_=e[:, cs:cs + P],
                            compare_op=mybir.AluOpType.is_ge, fill=0.0,
                            base=0, pattern=[[1, P]], channel_multiplier=-1,
                        )
                for h in range(HPC):
                    nc.tensor.matmul(
                        pc[h][:, cs:QG],
                        v[:, kt, 65 * h:65 * h + 65],
                        es[h][:, cs:QG],
                        start=(kt == 0), stop=(kt == nkt - 1),
                    )

            # normalize: ctxT[64h:64h+64, qg] = ctx / sums
            for h in range(HPC):
                sums_sb = sums_pool.tile([DH + 1, QG], F32R, tag="sums", name="sums")
                # ACT (exp) is the attention-phase bottleneck; keep the
                # softmax-sum row copies on DVE
                nc.vector.tensor_copy(sums_sb[DH:DH + 1, :], pc[h][DH:DH + 1, :])
                psb = ps_big.tile([DH, QG], F32, tag="ps_big", name="psb")
                nc.tensor.matmul(
                    psb[:], ones_sb[DH:DH + 1, 0:DH], sums_sb[DH:DH + 1, :],
                    start=True, stop=True, tile_position=(64, 0),
                )
                rcp = rcp_pool.tile([DH, QG], F32, tag="rcp", name="rcp")
                nc.vector.reciprocal_approx_fast(rcp[:], psb[:])
                nc.vector.tensor_tensor(
                    ctxT[DH * h:DH * h + DH, qg * QG:(qg + 1) * QG],
                    pc[h][0:DH, :], rcp[:],
                    mybir.AluOpType.mult,
                )

            # out-projection for the q tiles of this group
            for qt in range(4 * qg, 4 * qg + 4):
                o_sb = o_pool.tile([P, D], F32, tag="o", name="o_sb")
                for half in range(2):
                    pso = ps_big.tile([P, QG], F32, tag="ps_big", name="pso")
                    nc.tensor.matmul(
                        pso[:],
                        ctxT[:, qt * P:(qt + 1) * P],
                        wo_sb[:, half * QG:(half + 1) * QG],
                        start=True, stop=True,
                    )
                    if half == 1 and (o_copy_split or (b == B - 1 and qg == NQG - 1)):
                        # ACT is idle at the kernel tail; offload half the
                        # final output copies there
                        nc.scalar.copy(o_sb[:, half * QG:(half + 1) * QG], pso[:])
                    else:
                        nc.vector.tensor_copy(
                            o_sb[:, half * QG:(half + 1) * QG], pso[:])
                    # store each half as soon as its copy lands
                    nc.sync.dma_start(
                        out[b, qt * P:(qt + 1) * P, half * QG:(half + 1) * QG],
                        o_sb[:, half * QG:(half + 1) * QG])

        for step in schedule:
            if step[0] == "qkv":
                emit_qkv(step[1])
            else:
                emit_attn(step[1], step[2])

    nc.compile()
    return nc


_CACHE: dict = {}


def _get_nc() -> bass.Bass:
    if "nc" not in _CACHE:
        _CACHE["nc"] = build_nc()
    return _CACHE["nc"]


def make_in_maps(X, Wq, Wk, Wv, Wo):
    xbf = np.ascontiguousarray(
        np.asarray(X, dtype=np.float32).transpose(0, 2, 1)
    ).astype(ml_dtypes.bfloat16)
    in_maps = []
    for c in range(NCORES):
        sl = slice(c * DPC, (c + 1) * DPC)
        in_maps.append({
            "x": xbf,
            "wq": np.ascontiguousarray(Wq[:, sl]).astype(ml_dtypes.bfloat16),
            "wk": np.ascontiguousarray(Wk[:, sl]).astype(ml_dtypes.bfloat16),
            "wv": np.ascontiguousarray(Wv[:, sl]).astype(ml_dtypes.bfloat16),
            "wo": np.ascontiguousarray(Wo[sl, :]).astype(np.float32),
        })
    return in_maps


def run_spmd(X, Wq, Wk, Wv, Wo, bo, **run_kwargs):
    nc = _get_nc()
    in_maps = make_in_maps(X, Wq, Wk, Wv, Wo)
    res = run_bass_kernel_spmd(nc, in_maps, core_ids=list(range(NCORES)), **run_kwargs)
    acc = np.zeros((B, N, D), dtype=np.float32)
    for r in res.results:
        acc += r["out"]
    acc += np.asarray(bo, dtype=np.float32)
    return acc, res


def kernel(X, Wq, Wk, Wv, Wo, bo):
    out, _ = run_spmd(X, Wq, Wk, Wv, Wo, bo)
    return out

